# revision 40
# baseline (speedup 1.0000x reference)
"""Trainium2 Bass kernel for nn_Net_24429773979977 (dense_mlp).

Computes: 3-layer MLP over [B,T,D]=[2048,128,128] -> f [N,64], row-normalize
u = f/max(||f||,eps), return (||sum u||^2 - sum|u|^2) / (2N).

Strategy (data-parallel over 8 cores, 32768 rows per core):
 - Host pre-transposes x to feature-major xT [128, N] bf16; strided per-pair
   DMA (measured faster than a blocked contiguous DRAM layout on this HBM).
 - Pair-level software pipeline (1024 rows per iteration, two 512-row tiles
   packed onto 128 partitions) with multi-stage skew so every cross-engine
   dependency has >= 1 iteration of slack and the PE rarely idles (idling
   drops its p-state from 2.4 to 1.2 GHz, almost doubling matmul time):
     iter p: x-DMA(p+4) | L1(p)+relu1(p) | L2(p-1)+relu2(p-1) |
             L3(p-2)+fev(p-2)+square(p-2) | ones(p-4) [+rsqrt+stt_a at
             group boundary] | stt_b(p-5)
 - relu engines alternate by pair parity (one ACT + one DVE relu per
   iteration; a pair's whole chain stays on one engine, which keeps its
   intra-chain deps in program order instead of cross-engine semaphores).
 - b3 is folded into L3 via a constant ones-row on h2 (stationary [73,64]
   = [W3^T; b3]), so p3 = f and the tail ops are bias-free.
 - Tail per pair: f evicted PSUM->SBUF bf16 (ACT Copy / DVE cast by
   parity), gpsimd squares it (tensor_tensor is the only fast SBUF op the
   Pool engine has; it cannot touch PSUM at all), block-diag ones-matmul
   broadcasts nsq to both packed halves, ACT rsqrt(+eps^2) per 2-pair
   group, DVE stt u=f*w with accum_out giving per-feature row sums.
 - PSUM: ps1 [96,1024] + ps2 [72,1024] (bufs=1 each) + ps3 [128,512]
   (bufs=2) + psn [128,1024] (bufs=1) = exactly 8 banks.
 - Host combines per-core partial sums (S) to the final scalar; sum(u*u)
   equals N to fp64 precision since all row norms here are >> eps.
 - Empirically the emission order and engine phase assignment are a sharp
   local optimum: reorderings of the per-iteration blocks, moving the
   f-eviction to one engine, or global 70/30 relu splits all measured
   10-20% slower despite identical aggregate engine loads.
"""

import os
from contextlib import ExitStack

import numpy as np

B, T, D = 2048, 128, 128
N = B * T
NCORES = 8
NC_ROWS = N // NCORES          # 32768 rows per core
TILE = 512                     # rows per matmul tile (PSUM bank = 512 fp32)
PAIR_ROWS = 2 * TILE           # two tiles packed into 128 partitions
NPAIRS = NC_ROWS // PAIR_ROWS  # 32
NGROUPS = NPAIRS // 2          # wide-tail groups of 2 pairs
H1, H2, H3 = 96, 72, 64
EPS = 1e-8
ARSQRT_FUNC = "Abs_reciprocal_sqrt"


def build_nc():
    import concourse.tile as tile
    from concourse import bacc, mybir

    f32 = mybir.dt.float32
    bf16 = mybir.dt.bfloat16

    nc = bacc.Bacc("TRN2", target_bir_lowering=False, debug=False)

    xT = nc.declare_dram_parameter("xT", [D, NC_ROWS], bf16, isOutput=False)
    w1t = nc.declare_dram_parameter("w1t", [D, H1], bf16, isOutput=False)
    w2t = nc.declare_dram_parameter("w2t", [H1, H2], bf16, isOutput=False)
    w3b = nc.declare_dram_parameter("w3b", [H2 + 1, H3], bf16, isOutput=False)
    onesbd = nc.declare_dram_parameter("onesbd", [128, 128], bf16, isOutput=False)
    b1 = nc.declare_dram_parameter("b1", [H1, 1], f32, isOutput=False)
    b2 = nc.declare_dram_parameter("b2", [H2, 1], f32, isOutput=False)
    epsv = nc.declare_dram_parameter("epsv", [128, 1], f32, isOutput=False)

    s_out = nc.declare_dram_parameter("s_out", [128, NPAIRS], f32, isOutput=True)

    add = mybir.AluOpType.add
    mult = mybir.AluOpType.mult
    amax = mybir.AluOpType.max

    with tile.TileContext(nc) as tc, ExitStack() as ctx:
        consts = ctx.enter_context(tc.tile_pool(name="consts", bufs=1))
        xpool = ctx.enter_context(tc.tile_pool(name="x", bufs=6))
        h1pool = ctx.enter_context(tc.tile_pool(name="h1", bufs=3))
        h2pool = ctx.enter_context(tc.tile_pool(name="h2", bufs=1))
        fpool = ctx.enter_context(tc.tile_pool(name="fsb", bufs=5))
        fsqpool = ctx.enter_context(tc.tile_pool(name="fsq", bufs=4))
        nbpool = ctx.enter_context(tc.tile_pool(name="nb", bufs=2))
        upool = ctx.enter_context(tc.tile_pool(name="u", bufs=1))
        scolpool = ctx.enter_context(tc.tile_pool(name="scol", bufs=1))
        ps1 = ctx.enter_context(tc.tile_pool(name="ps1", bufs=1, space="PSUM"))
        ps2 = ctx.enter_context(tc.tile_pool(name="ps2", bufs=1, space="PSUM"))
        ps3 = ctx.enter_context(tc.tile_pool(name="ps3", bufs=2, space="PSUM"))
        psn = ctx.enter_context(tc.tile_pool(name="psn", bufs=1, space="PSUM"))

        w1_sb = consts.tile([D, H1], bf16, tag="w1")
        nc.sync.dma_start(out=w1_sb[:], in_=w1t[:])
        x_first = []
        for _pf in range(3):
            _xt = xpool.tile([D, PAIR_ROWS], bf16, tag="xt", name=f"xt_pre{_pf}")
            nc.sync.dma_start(
                out=_xt[:], in_=xT[:, _pf * PAIR_ROWS:(_pf + 1) * PAIR_ROWS])
            x_first.append(_xt)
        w2_sb = consts.tile([H1, H2], bf16, tag="w2")
        nc.sync.dma_start(out=w2_sb[:], in_=w2t[:])
        w3b_sb = consts.tile([H2 + 1, H3], bf16, tag="w3b")
        nc.sync.dma_start(out=w3b_sb[:], in_=w3b[:])
        ones_sb = consts.tile([128, 128], bf16, tag="ones")
        nc.sync.dma_start(out=ones_sb[:], in_=onesbd[:])
        b1_sb = consts.tile([H1, 1], f32, tag="b1")
        nc.sync.dma_start(out=b1_sb[:], in_=b1[:])
        b2_sb = consts.tile([H2, 1], f32, tag="b2")
        nc.sync.dma_start(out=b2_sb[:], in_=b2[:])
        eps_sb = consts.tile([128, 1], f32, tag="epsv")
        nc.sync.dma_start(out=eps_sb[:], in_=epsv[:])

        scol = scolpool.tile([128, NPAIRS], f32, tag="scol")

        h2_tiles = []
        for i in range(4):
            h2t = h2pool.tile([H2 + 1, PAIR_ROWS], bf16, tag=f"h2_{i}")
            nc.vector.memset(h2t[:], 1.0)
            h2_tiles.append(h2t)

        u_scr = upool.tile([128, TILE], bf16, tag="u")

        arsqrt = getattr(mybir.ActivationFunctionType, ARSQRT_FUNC)
        Relu = mybir.ActivationFunctionType.Relu
        Copy = mybir.ActivationFunctionType.Copy

        xts = {}       # pair -> xt tile
        h1s = {}       # pair -> h1 tile
        p1s = {}       # pair -> ps1 tile
        p2s = {}       # pair -> ps2 tile
        p3s = {}       # group -> wide ps3 tile
        fsbs = {}      # group -> wide f_sb tile
        fsqs = {}      # group -> wide fsq tile
        pns = {}       # group -> wide psn tile
        nbs = {}       # group -> wide nb tile

        def dma_x(p):
            xt = xpool.tile([D, PAIR_ROWS], bf16, tag="xt")
            nc.sync.dma_start(
                out=xt[:], in_=xT[:, p * PAIR_ROWS:(p + 1) * PAIR_ROWS])
            xts[p] = xt

        def l1(p):
            p1 = ps1.tile([H1, PAIR_ROWS], f32, tag="ps1")
            xt = xts[p]
            nc.tensor.matmul(p1[:, 0:TILE], w1_sb[:], xt[:, 0:TILE], start=True, stop=True)
            nc.tensor.matmul(p1[:, TILE:PAIR_ROWS], w1_sb[:], xt[:, TILE:PAIR_ROWS], start=True, stop=True)
            p1s[p] = p1

        def relu1(p):
            h1t = h1pool.tile([H1, PAIR_ROWS], bf16, tag="h1")
            if p % 2 == 0:
                nc.scalar.activation(h1t[:], p1s[p][:], Relu, bias=b1_sb[:], scale=1.0)
            else:
                nc.vector.tensor_scalar(h1t[:], p1s[p][:], b1_sb[:], 0.0,
                                        op0=add, op1=amax)
            h1s[p] = h1t
            del p1s[p]

        def l2(p):
            p2 = ps2.tile([H2, PAIR_ROWS], f32, tag="ps2")
            h1t = h1s[p]
            nc.tensor.matmul(p2[:, 0:TILE], w2_sb[:], h1t[:, 0:TILE], start=True, stop=True)
            nc.tensor.matmul(p2[:, TILE:PAIR_ROWS], w2_sb[:], h1t[:, TILE:PAIR_ROWS], start=True, stop=True)
            p2s[p] = p2
            del h1s[p]

        def relu2(p):
            h2t = h2_tiles[p % 4]
            if p % 2 == 0:
                nc.scalar.activation(h2t[0:H2, :], p2s[p][:], Relu, bias=b2_sb[:], scale=1.0)
            else:
                nc.vector.tensor_scalar(h2t[0:H2, :], p2s[p][:], b2_sb[:], 0.0,
                                        op0=add, op1=amax)
            del p2s[p]

        def l3(p):
            p3 = ps3.tile([128, TILE], f32, tag="ps3", name="p3t")
            h2t = h2_tiles[p % 4]
            nc.tensor.matmul(p3[0:H3, :], w3b_sb[:],
                             h2t[:, 0:TILE], start=True, stop=True)
            nc.tensor.matmul(p3[H3:128, :], w3b_sb[:],
                             h2t[:, TILE:PAIR_ROWS], start=True, stop=True)
            p3s[p] = p3

        Square = mybir.ActivationFunctionType.Square

        def fev_sq(p):
            fsq = fsqpool.tile([128, TILE], bf16, tag="fsq")
            if p >= NPAIRS - 2:
                # final group: no later L3 needs ps3, so keep p3 alive,
                # square straight from PSUM and let stt read PSUM too
                nc.scalar.activation(fsq[:], p3s[p][:], Square, bias=0.0, scale=1.0)
                fsqs[p] = fsq
                return
            f_sb = fpool.tile([128, TILE], bf16, tag="fsb")
            if p % 2 == 0:
                nc.scalar.activation(f_sb[:], p3s[p][:], Copy, bias=0.0, scale=1.0)
            else:
                nc.vector.tensor_copy(f_sb[:], p3s[p][:])
            fsbs[p] = f_sb
            del p3s[p]
            nc.gpsimd.tensor_tensor(fsq[:], f_sb[:], f_sb[:], op=mult)
            fsqs[p] = fsq

        def ones_mm(p):
            g, half = p // 2, p % 2
            if half == 0:
                pns[g] = psn.tile([128, PAIR_ROWS], f32, tag="psn", name="pnw")
            pn = pns[g]
            off = half * TILE
            nc.tensor.matmul(pn[:, off:off + TILE], ones_sb[:],
                             fsqs[p][:], start=True, stop=True)
            del fsqs[p]

        nbs = {}

        def _fsrc(p):
            return p3s[p] if p >= NPAIRS - 2 else fsbs.pop(p)

        def rsqrt_stt_a(g):
            nb = nbpool.tile([128, PAIR_ROWS], bf16, tag="nb")
            if g == NGROUPS - 1:
                nc.scalar.activation(nb[:, 0:TILE], pns[g][:, 0:TILE],
                                     arsqrt, bias=eps_sb[:], scale=1.0)
            else:
                nc.scalar.activation(nb[:], pns[g][:], arsqrt, bias=eps_sb[:], scale=1.0)
                del pns[g]
            nbs[g] = nb
            nc.vector.scalar_tensor_tensor(
                u_scr[:], _fsrc(2 * g)[:], 1.0, nb[:, 0:TILE],
                op0=mult, op1=mult, accum_out=scol[:, 2 * g:2 * g + 1])

        def stt_b(g):
            if g == NGROUPS - 1:
                nc.scalar.activation(nbs[g][:, TILE:PAIR_ROWS],
                                     pns[g][:, TILE:PAIR_ROWS],
                                     arsqrt, bias=eps_sb[:], scale=1.0)
                del pns[g]
            nc.vector.scalar_tensor_tensor(
                u_scr[:], _fsrc(2 * g + 1)[:], 1.0, nbs[g][:, TILE:PAIR_ROWS],
                op0=mult, op1=mult, accum_out=scol[:, 2 * g + 1:2 * g + 2])
            del nbs[g]

        for _pf in range(3):
            xts[_pf] = x_first[_pf]
        dma_x(3)
        LAST = NPAIRS + 6
        for p in range(LAST + 1):
            if p + 4 < NPAIRS:
                dma_x(p + 4)
            if p < NPAIRS:
                l1(p)
                relu1(p)
            if 0 <= p - 1 < NPAIRS:
                l2(p - 1)
                relu2(p - 1)
            if 0 <= p - 2 < NPAIRS:
                l3(p - 2)
                fev_sq(p - 2)
            if 0 <= p - 4 < NPAIRS:
                ones_mm(p - 4)
                if (p - 4) % 2 == 1:
                    rsqrt_stt_a((p - 4) // 2)
            if 0 <= p - 5 < NPAIRS and (p - 5) % 2 == 1:
                stt_b((p - 5) // 2)

        nc.sync.dma_start(out=s_out[:], in_=scol[:])

    nc.compile()
    return nc


def _prep_host_inputs(x, W1, b1, W2, b2, W3, b3):
    import ml_dtypes

    bf = ml_dtypes.bfloat16
    xflat = np.ascontiguousarray(x.reshape(N, D))
    in_maps = []
    w1t = np.ascontiguousarray(W1.T).astype(bf)
    w2t = np.ascontiguousarray(W2.T).astype(bf)
    w3b = np.concatenate([W3.T, b3.reshape(1, H3)], axis=0).astype(bf)
    onesbd = np.zeros((128, 128), np.float32)
    onesbd[:H3, :H3] = 1.0
    onesbd[H3:, H3:] = 1.0
    onesbd = onesbd.astype(bf)
    b1c = np.ascontiguousarray(b1.reshape(H1, 1), dtype=np.float32)
    b2c = np.ascontiguousarray(b2.reshape(H2, 1), dtype=np.float32)
    for c in range(NCORES):
        xT_c = np.ascontiguousarray(
            xflat[c * NC_ROWS:(c + 1) * NC_ROWS].T
        ).astype(bf)
        in_maps.append({
            "xT": xT_c, "w1t": w1t, "w2t": w2t, "w3b": w3b,
            "onesbd": onesbd, "b1": b1c, "b2": b2c,
            "epsv": np.full((128, 1), EPS * EPS, np.float32),
        })
    return in_maps


def _combine(results):
    """results: list of per-core dicts with s_out [128, NPAIRS]."""
    S = np.zeros(H3, np.float64)
    nrows = 0
    for r in results:
        sc = np.asarray(r["s_out"], np.float64)
        S += sc[:H3].sum(axis=1) + sc[H3:128].sum(axis=1)
        nrows += sc.shape[1] * PAIR_ROWS
    pair = 0.5 * (S @ S - float(nrows))
    return np.float32(pair / N)


_NC_CACHE = {}


def kernel(x, W1, b1, W2, b2, W3, b3):
    from concourse.bass_utils import run_bass_kernel_spmd

    if "nc" not in _NC_CACHE:
        _NC_CACHE["nc"] = build_nc()
    nc = _NC_CACHE["nc"]
    in_maps = _prep_host_inputs(
        np.asarray(x, np.float32), np.asarray(W1, np.float32),
        np.asarray(b1, np.float32), np.asarray(W2, np.float32),
        np.asarray(b2, np.float32), np.asarray(W3, np.float32),
        np.asarray(b3, np.float32),
    )
    res = run_bass_kernel_spmd(nc, in_maps, list(range(NCORES)))
    return _combine(res.results)


if __name__ == "__main__":
    pass
